# revision 6
# baseline (speedup 1.0000x reference)
"""Trainium2 Bass kernel for nn_CircularBottleneck (3x masked circular conv + BN + lrelu + residual).

Strategy: each circular conv with its (17x17-masked) kernel folds to a 15x15
circular kernel on the 16x16 torus (masks never touch kernel row/col 0, 8, 16
after folding -> 15 active column shifts).  For each column shift b we express
the conv as dense matmuls with host-precomputed circulant weights:

    out[(o,i), (n,j)] += W_b[(c,r), (o,i)] @ x_rot_b[(c,r), (n,j)]

with W_b[(c,r),(o,i)] = weff[o, c, (r-i)%16, b] and x_rot_b a j-rotation of x.
Batch (256) is sharded over 8 NeuronCores; BN statistics are all-reduced
on-device (tiny [27|81]x2 buffers).  Matmuls run in float32r (full PE rate at
N=512, ~2e-4 rounding).
"""
import os
import sys
import types

import numpy as np

sys.path.insert(0, "/opt/trn_rl_repo")
os.environ.setdefault("MYCRO_LOCAL_CACHE", "1")

# ---------------------------------------------------------------- problem spec
SIZE = 16
INPLANES = 81
E1, E2 = 1, 3
EPS = 1e-5
SLOPE = 0.01
B = 256
NCORES = 8
NB = B // NCORES
F = NB * 16  # free dim per matmul: (n, j) = 512
NTOT = B * SIZE * SIZE  # BN reduction count per channel


def _config_list(size, stride):
    rec = []
    s = stride
    for p in range(1, size // 2):
        for d in range(1, 2 * p + s - 1 + 16):
            if (2 * p + s - 1) % d != 0:
                continue
            rec.append((d, (2 * p + s - 1) // d + 1))
    return rec


def _build_mask(out_c, in_c, size, cfgs):
    ms = []
    for d, k in cfgs:
        m = np.zeros((out_c, in_c, size + 1, size + 1), dtype=np.float32)
        k_eff = (k - 1) * (d - 1) + k
        st = (size - k_eff) // 2 + 1
        en = (size + k_eff) // 2 + 1
        m[..., st:en:d, st:en:d] = 1.0
        ms.append(m)
    return np.concatenate(ms, axis=0)


CFGS = _config_list(SIZE, 1)
NCFG = len(CFGS)  # 27
B_LIST = [b for b in range(16) if b != 8]  # folded col 8 is always zero


def _fold16(wm):
    out = np.zeros((*wm.shape[:2], 16, 16), np.float32)
    for u in range(17):
        for v in range(17):
            out[:, :, (u - 8) % 16, (v - 8) % 16] += wm[:, :, u, v]
    return out


def _build_blob1(weff):
    """[15, 128, 11*432]: per-b block rows (c_l,r) c=8kt+c_l (c pad to 88),
    cols kt*432 + g*108 + o*4 + il  (i = 4g+il)."""
    Co, Ci = weff.shape[:2]
    wp = np.zeros((Co, 88, 16, 16), np.float32)
    wp[:, :Ci] = weff
    rot = (np.arange(16)[:, None] - np.arange(16)[None, :]) % 16  # [r, i]
    arr = wp[:, :, rot, :]  # [o, c, r, i, b16]
    arr = arr.transpose(4, 1, 2, 0, 3)[B_LIST]  # [15, 88, 16, 27, 16]
    arr = arr.reshape(15, 11, 128, 27, 4, 4).transpose(0, 1, 2, 4, 3, 5)
    return np.ascontiguousarray(arr.reshape(15, 11, 128, 432).transpose(0, 2, 1, 3)
                                ).reshape(15, 128, 11 * 432)


def _build_blob23(weff, n_mg):
    """[15, 128, 4*n_mg*108]: rows p=(c,il2) (slab s: r=4s+il2, rows pad to 128),
    cols s*(n_mg*108) + mg*108 + o_l*4 + il  (mg=(ob,ib), o=ob*27+o_l, i=4ib+il)."""
    Co, Ci = weff.shape[:2]
    rot = (np.arange(16)[:, None] - np.arange(16)[None, :]) % 16
    arr = weff[:, :, rot, :]  # [o, c, r, i, b16]
    arr = arr.transpose(4, 1, 2, 0, 3)[B_LIST]  # [15, c, 16, o, 16]
    arr = arr.reshape(15, Ci, 4, 4, Co // 27, 27, 4, 4)
    arr = arr.transpose(0, 2, 1, 3, 4, 6, 5, 7)  # [15, s, c, il2, ob, ib, o_l, il]
    arr = arr.reshape(15, 4, Ci * 4, n_mg * 108)
    blob = np.zeros((15, 4, 128, n_mg * 108), np.float32)
    blob[:, :, : Ci * 4] = arr
    return np.ascontiguousarray(blob.transpose(0, 2, 1, 3)).reshape(
        15, 128, 4 * n_mg * 108
    )


def _x_to_slabs(x):
    """(NB, 81, 16, 16) -> [128, 11*512] (p=(c_l,r), free = kt*512 + n*16 + j)."""
    xp = np.zeros((NB, 88, 16, 16), np.float32)
    xp[:, :81] = x
    t = xp.transpose(1, 2, 0, 3).reshape(88, 16, F).reshape(11, 128, F)
    return np.ascontiguousarray(t.transpose(1, 0, 2)).reshape(128, 11 * F)


def _x_to_bankunits(x):
    """(NB, 81, 16, 16) -> [108, 12*512]: bu=(ob,ib), p=o_l*4+il (i=4ib+il)."""
    t = x.reshape(NB, 3, 27, 4, 4, 16)  # n, ob, o_l, ib, il, j
    t = t.transpose(2, 4, 1, 3, 0, 5)  # o_l, il, ob, ib, n, j
    return np.ascontiguousarray(t.reshape(108, 12, F).transpose(0, 1, 2)).reshape(
        108, 12 * F
    ).copy()


_CACHE = {}


def _build_program():
    import concourse.bass as bass  # noqa: PLC0415
    import concourse.tile as tile  # noqa: PLC0415
    from concourse import bacc, mybir  # noqa: PLC0415

    f32 = mybir.dt.float32
    f32r = mybir.dt.float32r

    nc = bacc.Bacc("TRN2", target_bir_lowering=False, debug=False, num_devices=NCORES)

    xs_d = nc.dram_tensor("xs", [128, 11 * F], f32, kind="ExternalInput")
    xres_d = nc.dram_tensor("xres", [108, 12 * F], f32, kind="ExternalInput")
    wb1_d = nc.dram_tensor("wb1", [15, 128, 11 * 432], f32r, kind="ExternalInput")
    wb2_d = nc.dram_tensor("wb2", [15, 128, 4 * 432], f32r, kind="ExternalInput")
    wb3_d = nc.dram_tensor("wb3", [15, 128, 4 * 1296], f32r, kind="ExternalInput")
    sel_d = nc.dram_tensor("sel", [108, 27], f32, kind="ExternalInput")
    bc_d = nc.dram_tensor("bc", [27, 108], f32, kind="ExternalInput")
    gb_d = nc.dram_tensor("gb", [27, 10], f32, kind="ExternalInput")
    y_d = nc.dram_tensor("yout", [108, 12 * F], f32, kind="ExternalOutput")

    with tile.TileContext(nc) as tc:
        with (
            tc.tile_pool(name="persist", bufs=1) as pp,
            tc.tile_pool(name="xb", bufs=2) as xbp,
            tc.tile_pool(name="wb", bufs=2) as wbp,
            tc.tile_pool(name="work", bufs=2) as wk,
            tc.tile_pool(name="psum", bufs=4, space="PSUM") as psp,
            tc.tile_pool(name="dram", bufs=1, space="DRAM") as dram,
        ):
            xs = pp.tile([128, 11 * F], f32)
            xres = pp.tile([108, 12 * F], f32)
            sel = pp.tile([108, 27], f32)
            bc = pp.tile([27, 108], f32)
            gb = pp.tile([27, 10], f32)
            nc.sync.dma_start(xs[:], xs_d[:])
            nc.sync.dma_start(xres[:], xres_d[:])
            nc.sync.dma_start(sel[:], sel_d[:])
            nc.sync.dma_start(bc[:], bc_d[:])
            nc.sync.dma_start(gb[:], gb_d[:])

            def conv_pass(src, src_parts, kts, w_dram, wcols, mgs, psum_tiles):
                """Accumulate psum_tiles[i] (one per mg in mgs) over all 15 b."""
                n_src_cols = kts * F
                for bi in range(15):
                    b = B_LIST[bi]
                    wb = wbp.tile([128, wcols], f32r, tag="wb")
                    nc.sync.dma_start(wb[:], w_dram[bi])
                    xb = xbp.tile([src_parts, n_src_cols], f32r, tag="xb")
                    xv = src[0:src_parts, 0:n_src_cols].rearrange(
                        "p (kt n j) -> p kt n j", kt=kts, j=16
                    )
                    rv = xb[:].rearrange("p (kt n j) -> p kt n j", kt=kts, j=16)
                    nc.vector.tensor_copy(rv[:, :, :, 0 : 16 - b], xv[:, :, :, b:16])
                    if b:
                        nc.vector.tensor_copy(
                            rv[:, :, :, 16 - b : 16], xv[:, :, :, 0:b]
                        )
                    mcols = wcols // kts
                    for kt in range(kts):
                        rhs = xb[:, kt * F : (kt + 1) * F]
                        for i, mg in enumerate(mgs):
                            nc.tensor.matmul(
                                psum_tiles[i][:],
                                wb[0:src_parts, kt * mcols + mg * 108 : kt * mcols + (mg + 1) * 108],
                                rhs,
                                start=(bi == 0 and kt == 0),
                                stop=(bi == 14 and kt == kts - 1),
                            )

            def evac(psum_tiles, mgs, yraw, stats):
                for i, mg in enumerate(mgs):
                    ysl = yraw[:, mg * F : (mg + 1) * F]
                    nc.scalar.activation(
                        ysl,
                        psum_tiles[i][:],
                        mybir.ActivationFunctionType.Copy,
                        accum_out=stats[:, 2 * mg : 2 * mg + 1],
                    )
                    sq = wk.tile([108, F], f32, tag="sqd")
                    nc.scalar.activation(
                        sq[:],
                        ysl,
                        mybir.ActivationFunctionType.Square,
                        accum_out=stats[:, 2 * mg + 1 : 2 * mg + 2],
                    )

            def bn_scale_shift(stats, n_mg, n_ob, gcol, bcol, lname):
                """stats [108, 2*n_mg] -> scsh [108, 2*n_ob] (cols 2*ob+{0,1})."""
                ps_st = psp.tile([27, 2 * n_mg], f32, tag="cb")
                nc.tensor.matmul(
                    ps_st[:], sel[:], stats[:], start=True, stop=True
                )
                # st/ss/t/ss27 column storage is (stat, ob): col = s*n_ob + ob
                st = wk.tile([27, 2 * n_ob], f32, tag="st")
                # reduce over ib: cols of ps_st are 2*mg+stat, mg = ob*4+ib
                nc.vector.tensor_reduce(
                    st[:].rearrange("p (s ob) -> p ob s", ob=n_ob),
                    ps_st[:].rearrange("p (ob ib s) -> p ob s ib", ob=n_ob, s=2),
                    axis=mybir.AxisListType.X,
                    op=mybir.AluOpType.add,
                )
                arin = dram.tile([27, 2 * n_ob], f32, tag=f"arin{lname}")
                arout = dram.tile([27, 2 * n_ob], f32, tag=f"arout{lname}")
                nc.sync.dma_start(arin[:], st[:])
                nc.gpsimd.collective_compute(
                    "AllReduce",
                    mybir.AluOpType.add,
                    replica_groups=[list(range(NCORES))],
                    ins=[arin.opt()],
                    outs=[arout.opt()],
                )
                ss = wk.tile([27, 2 * n_ob], f32, tag="ss")
                nc.sync.dma_start(ss[:], arout[:])
                # mean/ex2 (cols: [0:n_ob]=sum-derived mean, [n_ob:]=ex2)
                t = wk.tile([27, 2 * n_ob], f32, tag="t")
                nc.scalar.mul(t[:], ss[:], 1.0 / NTOT)
                mean = t[:, 0:n_ob]
                ex2 = t[:, n_ob : 2 * n_ob]
                m2 = wk.tile([27, n_ob], f32, tag="m2")
                nc.vector.tensor_mul(m2[:], mean, mean)
                var = wk.tile([27, n_ob], f32, tag="var")
                nc.vector.tensor_sub(var[:], ex2, m2[:])
                vare = wk.tile([27, n_ob], f32, tag="vare")
                nc.vector.tensor_scalar_add(vare[:], var[:], EPS)
                sd = wk.tile([27, n_ob], f32, tag="sd")
                nc.scalar.activation(
                    sd[:], vare[:], mybir.ActivationFunctionType.Sqrt
                )
                rsd = wk.tile([27, n_ob], f32, tag="rsd")
                nc.vector.reciprocal(rsd[:], sd[:])
                ss27 = wk.tile([27, 2 * n_ob], f32, tag="ss27")
                scv = ss27[:, 0:n_ob]
                shv = ss27[:, n_ob : 2 * n_ob]
                nc.vector.tensor_mul(scv, rsd[:], gb[:, gcol : gcol + n_ob])
                ms = wk.tile([27, n_ob], f32, tag="ms")
                nc.vector.tensor_mul(ms[:], mean, scv)
                nc.vector.tensor_sub(shv, gb[:, bcol : bcol + n_ob], ms[:])
                ps_bc = psp.tile([108, 2 * n_ob], f32, tag="cb")
                nc.tensor.matmul(ps_bc[:], bc[:], ss27[:], start=True, stop=True)
                scsh = wk.tile([108, 2 * n_ob], f32, tag=f"scsh{lname}")
                nc.scalar.copy(scsh[:], ps_bc[:])
                return scsh

            # ---------------- layer 1 ----------------
            yraw1 = wk.tile([108, 4 * F], f32, tag="yraw")
            s1 = wk.tile([108, 8], f32, tag="stats")
            pt = [psp.tile([108, F], f32, tag="cb", name=f"ps1_{i}") for i in range(4)]
            conv_pass(xs, 128, 11, wb1_d, 11 * 432, [0, 1, 2, 3], pt)
            evac(pt, [0, 1, 2, 3], yraw1, s1)
            scsh1 = bn_scale_shift(s1, 4, 1, 0, 1, "l1")
            y1 = pp.tile([108, 4 * F], f32r)
            for mg in range(4):
                nc.scalar.activation(
                    y1[:, mg * F : (mg + 1) * F],
                    yraw1[:, mg * F : (mg + 1) * F],
                    mybir.ActivationFunctionType.Lrelu,
                    bias=scsh1[:, 1:2],
                    scale=scsh1[:, 0:1],
                    alpha=SLOPE,
                )

            # ---------------- layer 2 ----------------
            yraw2 = wk.tile([108, 4 * F], f32, tag="yraw")
            s2 = wk.tile([108, 8], f32, tag="stats")
            pt = [psp.tile([108, F], f32, tag="cb", name=f"ps2_{i}") for i in range(4)]
            conv_pass(y1, 108, 4, wb2_d, 4 * 432, [0, 1, 2, 3], pt)
            evac(pt, [0, 1, 2, 3], yraw2, s2)
            scsh2 = bn_scale_shift(s2, 4, 1, 2, 3, "l2")
            y2 = pp.tile([108, 4 * F], f32r)
            for mg in range(4):
                nc.scalar.activation(
                    y2[:, mg * F : (mg + 1) * F],
                    yraw2[:, mg * F : (mg + 1) * F],
                    mybir.ActivationFunctionType.Lrelu,
                    bias=scsh2[:, 1:2],
                    scale=scsh2[:, 0:1],
                    alpha=SLOPE,
                )

            # ---------------- layer 3 (3 passes of 4 bank-units) --------------
            yraw3 = pp.tile([108, 12 * F], f32)
            s3 = wk.tile([108, 24], f32, tag="stats")
            for pass_i in range(3):
                mgs = list(range(4 * pass_i, 4 * pass_i + 4))
                pt = [
                    psp.tile([108, F], f32, tag="cb", name=f"ps3_{pass_i}_{i}")
                    for i in range(4)
                ]
                conv_pass(y2, 108, 4, wb3_d, 4 * 1296, mgs, pt)
                evac(pt, mgs, yraw3, s3)
            scsh3 = bn_scale_shift(s3, 12, 3, 4, 7, "l3")
            for mg in range(12):
                ob = mg // 4
                bn_t = wk.tile([108, F], f32, tag="bn_t")
                nc.scalar.activation(
                    bn_t[:],
                    yraw3[:, mg * F : (mg + 1) * F],
                    mybir.ActivationFunctionType.Identity,
                    bias=scsh3[:, 3 + ob : 4 + ob],
                    scale=scsh3[:, ob : ob + 1],
                )
                sum_t = wk.tile([108, F], f32, tag="sum_t")
                nc.vector.tensor_add(
                    sum_t[:], bn_t[:], xres[:, mg * F : (mg + 1) * F]
                )
                out_t = wk.tile([108, F], f32, tag="out_t")
                nc.scalar.activation(
                    out_t[:], sum_t[:], mybir.ActivationFunctionType.Lrelu, alpha=SLOPE
                )
                nc.sync.dma_start(y_d[:, mg * F : (mg + 1) * F], out_t[:])

    nc.compile()
    return nc


def _get_program():
    if "nc" not in _CACHE:
        _CACHE["nc"] = _build_program()
    return _CACHE["nc"]


def kernel(x, w1, w2, w3, g1, b1, g2, b2, g3, b3):
    from concourse.bass_utils import run_bass_kernel_spmd  # noqa: PLC0415

    x = np.asarray(x, np.float32)
    m1 = _build_mask(E1, INPLANES, SIZE, CFGS)
    m2 = _build_mask(E1, NCFG * E1, SIZE, CFGS)
    m3 = _build_mask(E2, NCFG * E1, SIZE, CFGS)
    blob1 = _build_blob1(_fold16(np.asarray(w1, np.float32) * m1))
    blob2 = _build_blob23(_fold16(np.asarray(w2, np.float32) * m2), 4)
    blob3 = _build_blob23(_fold16(np.asarray(w3, np.float32) * m3), 12)

    sel = np.zeros((108, 27), np.float32)
    sel[np.arange(108), np.arange(108) // 4] = 1.0
    bc = np.ascontiguousarray(sel.T)

    gbm = np.zeros((27, 10), np.float32)
    gbm[:, 0] = np.asarray(g1, np.float32)
    gbm[:, 1] = np.asarray(b1, np.float32)
    gbm[:, 2] = np.asarray(g2, np.float32)
    gbm[:, 3] = np.asarray(b2, np.float32)
    gbm[:, 4:7] = np.asarray(g3, np.float32).reshape(3, 27).T
    gbm[:, 7:10] = np.asarray(b3, np.float32).reshape(3, 27).T

    nc = _get_program()
    in_maps = []
    for c in range(NCORES):
        shard = x[c * NB : (c + 1) * NB]
        in_maps.append(
            {
                "xs": _x_to_slabs(shard),
                "xres": _x_to_bankunits(shard),
                "wb1": blob1,
                "wb2": blob2,
                "wb3": blob3,
                "sel": sel,
                "bc": bc,
                "gb": gbm,
            }
        )
    res = run_bass_kernel_spmd(nc, in_maps, list(range(NCORES)), trace=False)

    out = np.zeros((B, 81, 16, 16), np.float32)
    for c in range(NCORES):
        yb = res.results[c]["yout"].reshape(108, 12, NB, 16)
        # [o_l*4+il, (ob, ib), n, j] -> [n, ob*27+o_l, 4*ib+il, j]
        t = yb.reshape(27, 4, 3, 4, NB, 16)  # o_l, il, ob, ib, n, j
        t = t.transpose(4, 2, 0, 3, 1, 5)  # n, ob, o_l, ib, il, j
        out[c * NB : (c + 1) * NB] = t.reshape(NB, 81, 16, 16)
    return out


# revision 7
# speedup vs baseline: 1.1650x; 1.1650x over previous
"""Trainium2 Bass kernel for nn_CircularBottleneck (3x masked circular conv + BN + lrelu + residual).

Strategy: each circular conv with its (17x17-masked) kernel folds to a 15x15
circular kernel on the 16x16 torus (masks never touch kernel row/col 0, 8, 16
after folding -> 15 active column shifts).  For each column shift b we express
the conv as dense matmuls with host-precomputed circulant weights:

    out[(o,i), (n,j)] += W_b[(c,r), (o,i)] @ x_rot_b[(c,r), (n,j)]

with W_b[(c,r),(o,i)] = weff[o, c, (r-i)%16, b] and x_rot_b a j-rotation of x.
Batch (256) is sharded over 8 NeuronCores; BN statistics are all-reduced
on-device (tiny [27|81]x2 buffers).  Matmuls run in float32r (full PE rate at
N=512, ~2e-4 rounding).
"""
import os
import sys
import types

import numpy as np

sys.path.insert(0, "/opt/trn_rl_repo")
os.environ.setdefault("MYCRO_LOCAL_CACHE", "1")

# ---------------------------------------------------------------- problem spec
SIZE = 16
INPLANES = 81
E1, E2 = 1, 3
EPS = 1e-5
SLOPE = 0.01
B = 256
NCORES = 8
NB = B // NCORES
F = NB * 16  # free dim per matmul: (n, j) = 512
NTOT = B * SIZE * SIZE  # BN reduction count per channel


def _config_list(size, stride):
    rec = []
    s = stride
    for p in range(1, size // 2):
        for d in range(1, 2 * p + s - 1 + 16):
            if (2 * p + s - 1) % d != 0:
                continue
            rec.append((d, (2 * p + s - 1) // d + 1))
    return rec


def _build_mask(out_c, in_c, size, cfgs):
    ms = []
    for d, k in cfgs:
        m = np.zeros((out_c, in_c, size + 1, size + 1), dtype=np.float32)
        k_eff = (k - 1) * (d - 1) + k
        st = (size - k_eff) // 2 + 1
        en = (size + k_eff) // 2 + 1
        m[..., st:en:d, st:en:d] = 1.0
        ms.append(m)
    return np.concatenate(ms, axis=0)


CFGS = _config_list(SIZE, 1)
NCFG = len(CFGS)  # 27
B_LIST = [b for b in range(16) if b != 8]  # folded col 8 is always zero


def _fold16(wm):
    out = np.zeros((*wm.shape[:2], 16, 16), np.float32)
    for u in range(17):
        for v in range(17):
            out[:, :, (u - 8) % 16, (v - 8) % 16] += wm[:, :, u, v]
    return out


def _build_blob1(weff):
    """[15, 128, 11*432]: per-b block rows (c_l,r) c=8kt+c_l (c pad to 88),
    cols kt*432 + g*108 + o*4 + il  (i = 4g+il)."""
    Co, Ci = weff.shape[:2]
    wp = np.zeros((Co, 88, 16, 16), np.float32)
    wp[:, :Ci] = weff
    rot = (np.arange(16)[:, None] - np.arange(16)[None, :]) % 16  # [r, i]
    arr = wp[:, :, rot, :]  # [o, c, r, i, b16]
    arr = arr.transpose(4, 1, 2, 0, 3)[B_LIST]  # [15, 88, 16, 27, 16]
    arr = arr.reshape(15, 11, 128, 27, 4, 4).transpose(0, 1, 2, 4, 3, 5)
    return np.ascontiguousarray(arr.reshape(15, 11, 128, 432).transpose(0, 2, 1, 3)
                                ).reshape(15, 128, 11 * 432)


def _build_blob23(weff, n_mg):
    """[15, 128, 4*n_mg*108]: rows p=(c,il2) (slab s: r=4s+il2, rows pad to 128),
    cols s*(n_mg*108) + mg*108 + o_l*4 + il  (mg=(ob,ib), o=ob*27+o_l, i=4ib+il)."""
    Co, Ci = weff.shape[:2]
    rot = (np.arange(16)[:, None] - np.arange(16)[None, :]) % 16
    arr = weff[:, :, rot, :]  # [o, c, r, i, b16]
    arr = arr.transpose(4, 1, 2, 0, 3)[B_LIST]  # [15, c, 16, o, 16]
    arr = arr.reshape(15, Ci, 4, 4, Co // 27, 27, 4, 4)
    arr = arr.transpose(0, 2, 1, 3, 4, 6, 5, 7)  # [15, s, c, il2, ob, ib, o_l, il]
    arr = arr.reshape(15, 4, Ci * 4, n_mg * 108)
    blob = np.zeros((15, 4, 128, n_mg * 108), np.float32)
    blob[:, :, : Ci * 4] = arr
    return np.ascontiguousarray(blob.transpose(0, 2, 1, 3)).reshape(
        15, 128, 4 * n_mg * 108
    )


def _x_to_slabs(x):
    """(NB, 81, 16, 16) -> [128, 11*512] (p=(c_l,r), free = kt*512 + n*16 + j)."""
    xp = np.zeros((NB, 88, 16, 16), np.float32)
    xp[:, :81] = x
    t = xp.transpose(1, 2, 0, 3).reshape(88, 16, F).reshape(11, 128, F)
    return np.ascontiguousarray(t.transpose(1, 0, 2)).reshape(128, 11 * F)


def _x_to_bankunits(x):
    """(NB, 81, 16, 16) -> [108, 12*512]: bu=(ob,ib), p=o_l*4+il (i=4ib+il)."""
    t = x.reshape(NB, 3, 27, 4, 4, 16)  # n, ob, o_l, ib, il, j
    t = t.transpose(2, 4, 1, 3, 0, 5)  # o_l, il, ob, ib, n, j
    return np.ascontiguousarray(t.reshape(108, 12, F).transpose(0, 1, 2)).reshape(
        108, 12 * F
    ).copy()


_CACHE = {}


def _build_program():
    import concourse.bass as bass  # noqa: PLC0415
    import concourse.tile as tile  # noqa: PLC0415
    from concourse import bacc, mybir  # noqa: PLC0415

    f32 = mybir.dt.float32
    f32r = mybir.dt.float32r

    nc = bacc.Bacc("TRN2", target_bir_lowering=False, debug=False, num_devices=NCORES)

    xs_d = nc.dram_tensor("xs", [128, 11 * F], f32, kind="ExternalInput")
    xres_d = nc.dram_tensor("xres", [108, 12 * F], f32, kind="ExternalInput")
    wb1_d = nc.dram_tensor("wb1", [15, 128, 11 * 432], f32r, kind="ExternalInput")
    wb2_d = nc.dram_tensor("wb2", [15, 128, 4 * 432], f32r, kind="ExternalInput")
    wb3_d = nc.dram_tensor("wb3", [15, 128, 4 * 1296], f32r, kind="ExternalInput")
    sel_d = nc.dram_tensor("sel", [108, 27], f32, kind="ExternalInput")
    bc_d = nc.dram_tensor("bc", [27, 108], f32, kind="ExternalInput")
    gb_d = nc.dram_tensor("gb", [27, 10], f32, kind="ExternalInput")
    y_d = nc.dram_tensor("yout", [108, 12 * F], f32, kind="ExternalOutput")

    with tile.TileContext(nc) as tc:
        with (
            tc.tile_pool(name="persist", bufs=1) as pp,
            tc.tile_pool(name="xb", bufs=2) as xbp,
            tc.tile_pool(name="wb", bufs=2) as wbp,
            tc.tile_pool(name="work", bufs=2) as wk,
            tc.tile_pool(name="psum", bufs=4, space="PSUM") as psp,
            tc.tile_pool(name="dram", bufs=1, space="DRAM") as dram,
        ):
            xs = pp.tile([128, 11 * F], f32)
            xres = pp.tile([108, 12 * F], f32)
            sel = pp.tile([108, 27], f32)
            bc = pp.tile([27, 108], f32)
            gb = pp.tile([27, 10], f32)
            nc.sync.dma_start(xs[:], xs_d[:])
            nc.sync.dma_start(xres[:], xres_d[:])
            nc.sync.dma_start(sel[:], sel_d[:])
            nc.sync.dma_start(bc[:], bc_d[:])
            nc.sync.dma_start(gb[:], gb_d[:])

            # warmup AllReduce: absorbs collective first-use latency while
            # conv1 keeps the PE busy
            wu_in = dram.tile([27, 2], f32, tag="wu_in")
            wu_out = dram.tile([27, 2], f32, tag="wu_out")
            wu_t = wk.tile([27, 2], f32, tag="wu_t")
            nc.vector.tensor_copy(wu_t[:], gb[:, 0:2])
            nc.sync.dma_start(wu_in[:], wu_t[:])
            nc.gpsimd.collective_compute(
                "AllReduce",
                mybir.AluOpType.add,
                replica_groups=[list(range(NCORES))],
                ins=[wu_in.opt()],
                outs=[wu_out.opt()],
            )

            def conv_pass(src, src_parts, kts, w_dram, wcols, mgs, psum_tiles):
                """Accumulate psum_tiles[i] (one per mg in mgs) over all 15 b.
                Streams only the mg-slice of the weight blob each pass."""
                n_src_cols = kts * F
                mcols = wcols // kts
                mg0 = mgs[0]
                acols = len(mgs) * 108  # active weight cols per kt
                for bi in range(15):
                    b = B_LIST[bi]
                    wb = wbp.tile([128, kts * acols], f32r, tag="wb")
                    nc.sync.dma_start(
                        wb[:].rearrange("p (kt m) -> p kt m", kt=kts),
                        w_dram[bi].rearrange("p (kt m) -> p kt m", kt=kts)[
                            :, :, mg0 * 108 : mg0 * 108 + acols
                        ],
                    )
                    xb = xbp.tile([src_parts, n_src_cols], f32r, tag="xb")
                    xv = src[0:src_parts, 0:n_src_cols].rearrange(
                        "p (kt n j) -> p kt n j", kt=kts, j=16
                    )
                    rv = xb[:].rearrange("p (kt n j) -> p kt n j", kt=kts, j=16)
                    nc.vector.tensor_copy(rv[:, :, :, 0 : 16 - b], xv[:, :, :, b:16])
                    if b:
                        nc.vector.tensor_copy(
                            rv[:, :, :, 16 - b : 16], xv[:, :, :, 0:b]
                        )
                    for kt in range(kts):
                        rhs = xb[:, kt * F : (kt + 1) * F]
                        for i, mg in enumerate(mgs):
                            nc.tensor.matmul(
                                psum_tiles[i][:],
                                wb[0:src_parts, kt * acols + (mg - mg0) * 108 : kt * acols + (mg - mg0 + 1) * 108],
                                rhs,
                                start=(bi == 0 and kt == 0),
                                stop=(bi == 14 and kt == kts - 1),
                            )

            def evac(psum_tiles, mgs, yraw, stats):
                for i, mg in enumerate(mgs):
                    ysl = yraw[:, mg * F : (mg + 1) * F]
                    nc.scalar.activation(
                        ysl,
                        psum_tiles[i][:],
                        mybir.ActivationFunctionType.Copy,
                        accum_out=stats[:, 2 * mg : 2 * mg + 1],
                    )
                    sq = wk.tile([108, F], f32, tag="sqd")
                    nc.scalar.activation(
                        sq[:],
                        ysl,
                        mybir.ActivationFunctionType.Square,
                        accum_out=stats[:, 2 * mg + 1 : 2 * mg + 2],
                    )

            def bn_scale_shift(stats, n_mg, n_ob, gcol, bcol, lname):
                """stats [108, 2*n_mg] -> scsh [108, 2*n_ob] (cols 2*ob+{0,1})."""
                ps_st = psp.tile([27, 2 * n_mg], f32, tag="cb")
                nc.tensor.matmul(
                    ps_st[:], sel[:], stats[:], start=True, stop=True
                )
                # st/ss/t/ss27 column storage is (stat, ob): col = s*n_ob + ob
                st = wk.tile([27, 2 * n_ob], f32, tag="st")
                # reduce over ib: cols of ps_st are 2*mg+stat, mg = ob*4+ib
                nc.vector.tensor_reduce(
                    st[:].rearrange("p (s ob) -> p ob s", ob=n_ob),
                    ps_st[:].rearrange("p (ob ib s) -> p ob s ib", ob=n_ob, s=2),
                    axis=mybir.AxisListType.X,
                    op=mybir.AluOpType.add,
                )
                arin = dram.tile([27, 2 * n_ob], f32, tag=f"arin{lname}")
                arout = dram.tile([27, 2 * n_ob], f32, tag=f"arout{lname}")
                nc.sync.dma_start(arin[:], st[:])
                nc.gpsimd.collective_compute(
                    "AllReduce",
                    mybir.AluOpType.add,
                    replica_groups=[list(range(NCORES))],
                    ins=[arin.opt()],
                    outs=[arout.opt()],
                )
                ss = wk.tile([27, 2 * n_ob], f32, tag="ss")
                nc.sync.dma_start(ss[:], arout[:])
                # mean/ex2 (cols: [0:n_ob]=sum-derived mean, [n_ob:]=ex2)
                t = wk.tile([27, 2 * n_ob], f32, tag="t")
                nc.scalar.mul(t[:], ss[:], 1.0 / NTOT)
                mean = t[:, 0:n_ob]
                ex2 = t[:, n_ob : 2 * n_ob]
                m2 = wk.tile([27, n_ob], f32, tag="m2")
                nc.vector.tensor_mul(m2[:], mean, mean)
                var = wk.tile([27, n_ob], f32, tag="var")
                nc.vector.tensor_sub(var[:], ex2, m2[:])
                vare = wk.tile([27, n_ob], f32, tag="vare")
                nc.vector.tensor_scalar_add(vare[:], var[:], EPS)
                sd = wk.tile([27, n_ob], f32, tag="sd")
                nc.scalar.activation(
                    sd[:], vare[:], mybir.ActivationFunctionType.Sqrt
                )
                rsd = wk.tile([27, n_ob], f32, tag="rsd")
                nc.vector.reciprocal(rsd[:], sd[:])
                ss27 = wk.tile([27, 2 * n_ob], f32, tag="ss27")
                scv = ss27[:, 0:n_ob]
                shv = ss27[:, n_ob : 2 * n_ob]
                nc.vector.tensor_mul(scv, rsd[:], gb[:, gcol : gcol + n_ob])
                ms = wk.tile([27, n_ob], f32, tag="ms")
                nc.vector.tensor_mul(ms[:], mean, scv)
                nc.vector.tensor_sub(shv, gb[:, bcol : bcol + n_ob], ms[:])
                ps_bc = psp.tile([108, 2 * n_ob], f32, tag="cb")
                nc.tensor.matmul(ps_bc[:], bc[:], ss27[:], start=True, stop=True)
                scsh = wk.tile([108, 2 * n_ob], f32, tag=f"scsh{lname}")
                nc.scalar.copy(scsh[:], ps_bc[:])
                return scsh

            # ---------------- layer 1 ----------------
            yraw1 = wk.tile([108, 4 * F], f32, tag="yraw")
            s1 = wk.tile([108, 8], f32, tag="stats")
            pt = [psp.tile([108, F], f32, tag="cb", name=f"ps1_{i}") for i in range(4)]
            conv_pass(xs, 128, 11, wb1_d, 11 * 432, [0, 1, 2, 3], pt)
            evac(pt, [0, 1, 2, 3], yraw1, s1)
            scsh1 = bn_scale_shift(s1, 4, 1, 0, 1, "l1")
            y1 = pp.tile([108, 4 * F], f32r)
            for mg in range(4):
                nc.scalar.activation(
                    y1[:, mg * F : (mg + 1) * F],
                    yraw1[:, mg * F : (mg + 1) * F],
                    mybir.ActivationFunctionType.Lrelu,
                    bias=scsh1[:, 1:2],
                    scale=scsh1[:, 0:1],
                    alpha=SLOPE,
                )

            # ---------------- layer 2 ----------------
            yraw2 = wk.tile([108, 4 * F], f32, tag="yraw")
            s2 = wk.tile([108, 8], f32, tag="stats")
            pt = [psp.tile([108, F], f32, tag="cb", name=f"ps2_{i}") for i in range(4)]
            conv_pass(y1, 108, 4, wb2_d, 4 * 432, [0, 1, 2, 3], pt)
            evac(pt, [0, 1, 2, 3], yraw2, s2)
            scsh2 = bn_scale_shift(s2, 4, 1, 2, 3, "l2")
            y2 = pp.tile([108, 4 * F], f32r)
            for mg in range(4):
                nc.scalar.activation(
                    y2[:, mg * F : (mg + 1) * F],
                    yraw2[:, mg * F : (mg + 1) * F],
                    mybir.ActivationFunctionType.Lrelu,
                    bias=scsh2[:, 1:2],
                    scale=scsh2[:, 0:1],
                    alpha=SLOPE,
                )

            # ---------------- layer 3 (3 passes of 4 bank-units) --------------
            yraw3 = pp.tile([108, 12 * F], f32)
            s3 = wk.tile([108, 24], f32, tag="stats")
            for pass_i in range(3):
                mgs = list(range(4 * pass_i, 4 * pass_i + 4))
                pt = [
                    psp.tile([108, F], f32, tag="cb", name=f"ps3_{pass_i}_{i}")
                    for i in range(4)
                ]
                conv_pass(y2, 108, 4, wb3_d, 4 * 1296, mgs, pt)
                evac(pt, mgs, yraw3, s3)
            scsh3 = bn_scale_shift(s3, 12, 3, 4, 7, "l3")
            for mg in range(12):
                ob = mg // 4
                bn_t = wk.tile([108, F], f32, tag="bn_t")
                nc.scalar.activation(
                    bn_t[:],
                    yraw3[:, mg * F : (mg + 1) * F],
                    mybir.ActivationFunctionType.Identity,
                    bias=scsh3[:, 3 + ob : 4 + ob],
                    scale=scsh3[:, ob : ob + 1],
                )
                sum_t = wk.tile([108, F], f32, tag="sum_t")
                nc.vector.tensor_add(
                    sum_t[:], bn_t[:], xres[:, mg * F : (mg + 1) * F]
                )
                out_t = wk.tile([108, F], f32, tag="out_t")
                nc.scalar.activation(
                    out_t[:], sum_t[:], mybir.ActivationFunctionType.Lrelu, alpha=SLOPE
                )
                nc.sync.dma_start(y_d[:, mg * F : (mg + 1) * F], out_t[:])

    nc.compile()
    return nc


def _get_program():
    if "nc" not in _CACHE:
        _CACHE["nc"] = _build_program()
    return _CACHE["nc"]


def kernel(x, w1, w2, w3, g1, b1, g2, b2, g3, b3):
    from concourse.bass_utils import run_bass_kernel_spmd  # noqa: PLC0415

    x = np.asarray(x, np.float32)
    m1 = _build_mask(E1, INPLANES, SIZE, CFGS)
    m2 = _build_mask(E1, NCFG * E1, SIZE, CFGS)
    m3 = _build_mask(E2, NCFG * E1, SIZE, CFGS)
    blob1 = _build_blob1(_fold16(np.asarray(w1, np.float32) * m1))
    blob2 = _build_blob23(_fold16(np.asarray(w2, np.float32) * m2), 4)
    blob3 = _build_blob23(_fold16(np.asarray(w3, np.float32) * m3), 12)

    sel = np.zeros((108, 27), np.float32)
    sel[np.arange(108), np.arange(108) // 4] = 1.0
    bc = np.ascontiguousarray(sel.T)

    gbm = np.zeros((27, 10), np.float32)
    gbm[:, 0] = np.asarray(g1, np.float32)
    gbm[:, 1] = np.asarray(b1, np.float32)
    gbm[:, 2] = np.asarray(g2, np.float32)
    gbm[:, 3] = np.asarray(b2, np.float32)
    gbm[:, 4:7] = np.asarray(g3, np.float32).reshape(3, 27).T
    gbm[:, 7:10] = np.asarray(b3, np.float32).reshape(3, 27).T

    nc = _get_program()
    in_maps = []
    for c in range(NCORES):
        shard = x[c * NB : (c + 1) * NB]
        in_maps.append(
            {
                "xs": _x_to_slabs(shard),
                "xres": _x_to_bankunits(shard),
                "wb1": blob1,
                "wb2": blob2,
                "wb3": blob3,
                "sel": sel,
                "bc": bc,
                "gb": gbm,
            }
        )
    res = run_bass_kernel_spmd(nc, in_maps, list(range(NCORES)), trace=False)

    out = np.zeros((B, 81, 16, 16), np.float32)
    for c in range(NCORES):
        yb = res.results[c]["yout"].reshape(108, 12, NB, 16)
        # [o_l*4+il, (ob, ib), n, j] -> [n, ob*27+o_l, 4*ib+il, j]
        t = yb.reshape(27, 4, 3, 4, NB, 16)  # o_l, il, ob, ib, n, j
        t = t.transpose(4, 2, 0, 3, 1, 5)  # n, ob, o_l, ib, il, j
        out[c * NB : (c + 1) * NB] = t.reshape(NB, 81, 16, 16)
    return out
